# revision 35
# baseline (speedup 1.0000x reference)
"""BiESN2D on 8 TRN2 NeuronCores (Bass/Tile).

Reference computes 4 directional leaky-tanh ESN scans over a (8,128,128,64)
image batch: horizontal fwd/bwd over rows, vertical fwd/bwd over columns,
each with U=256 units, outputs concatenated to (8,128,128,1024).

Sharding: core = (scan-direction, batch-half).  Each of the 8 cores runs ONE
scan type over S=512 sequences (4 batches x 128 rows/cols), T=128 steps.

Device recurrence (state kept transposed, u on partitions, s on free dim):
    w_t = 0.1*w_{t-1} + tanh(x_t @ Wk + w_{t-1} @ (0.9*Wr)),   h_t = 0.9*w_t
(the 0.9 scale and final layout transposes are applied on the host).

Structure per step, per s-chunk (3 independent recurrence chains pipeline
the PE -> ScalarE(tanh) -> VectorE(blend) serial path):
  - ONE accumulation group per chain-step, emitted with NO lookahead: the
    two K=64 x-matmuls OPEN it (their psum-free dep is tanh(t-2), ~2
    periods stale, so they never block the PE FIFO; lookahead-1 x-opens
    waited on tanh(t-1) at the FIFO head and cost ~300ns/step), then the
    4 K=128 rec matmuls follow with the group STOP on the last — the
    stop lands right at the end of the true serial path,
  - one ScalarE tanh (psum fp32 -> sbuf fp16, ~553ns for FD=352),
  - one VectorE scalar_tensor_tensor: w_new = 0.1*w_old + g into a
    16-slot SBUF ring (deep ring decouples the output DMA from the x
    prefetch; STT is 1x-mode ~515ns but a TS+TT split doesn't fit the
    DVE budget and GPSIMD elementwise is 5-10x too slow to help),
  - every 4 steps (pairs for the last 4), one batched DMA of ring slots
    to DRAM.
Steady state measured 1880ns/step, fully flat: the cycle is rec-MMs ~460
+ sem 57 + ACT ~545 + prop 31 + STT ~540 + queue ~250 (equalized 171/171/170
chunks shaved ~77ns/step off the binding chain vs 176/176/160).  NO per-step heater:
Tile schedules dep-free fillers greedily so they bunch into the early
steps (+350ns/step there), and the pipeline's natural matmul density
keeps the PE HAM clock-gate at K=8/8 (2.4GHz) for the whole run anyway;
only a small initial burst warms it up.  x input is DMA'd in 7 staged
chunks (small first chunk so compute starts ~11us in; later chunks
pushed from inside the loop so their semaphores never alias ahead of
compute).
NOTE: all matmuls of one accumulation group must use the SAME PE row half —
mixing row groups within one group crashes the hardware.
All compute except PSUM accumulation is fp16 (measured 5.0e-4 rel l2 error
vs the fp32 reference; scaled absmax 1.9e-3).
"""

import numpy as np
from contextlib import ExitStack

import concourse.bass as bass
import concourse.mybir as mybir
import concourse.tile as tile
from concourse import bacc
from concourse.bass_utils import run_bass_kernel_spmd

# ---------------- problem constants (hardcoded per spec) ----------------
B, NH, NW, C = 8, 128, 128, 64
U = 256           # units per directional ESN cell
T = 128           # scan length
S = 512           # sequences per core (4 batches * 128)
LEAKY = 0.9
N_CORES = 8

F16 = mybir.dt.float16
F32 = mybir.dt.float32

CHUNKS = (171, 171, 170)  # s-chunks; equalized so no chain's ACT/STT/rec
                          # streaming time dominates the latency-bound cycle
RING = 16                 # w-state ring slots per chain
DMA_BATCH = 4             # t-steps per output DMA
# x input DMA chunks: (start_t, len_t, push_at_iter). First chunks are small
# so compute can start ASAP; later chunks are pushed from inside the t-loop
# (pushing all upfront made the first x-matmul's semaphore alias a LATER
# chunk's DMA and stalled compute start by ~8us, and the y-output DMAs
# convoyed behind 12MB of queued x prefetch).
XCHUNKS = ((0, 8, -1), (8, 8, -1), (16, 16, -1), (32, 24, 8),
           (56, 24, 28), (80, 24, 52), (104, 24, 76))
HEAT_BURST = 4            # initial heater matmuls (N=512); the HAM flip
                          # completes during step 0's own matmul stream
# NO per-step heaters: Tile schedules dep-free fillers greedily, so they
# all bunch into the first ~60 steps (+350ns/step there), and the trace
# shows HAM stays at K=8/8 for the whole heater-free remainder anyway --
# the pipeline's natural matmul density (~75% PE busy) keeps the PE warm.
HEAT_PER_STEP = 0


def build_program(chunks=CHUNKS, t_steps=T, s_total=S,
                  heat_burst=HEAT_BURST, heat_per_step=HEAT_PER_STEP):
    """Build the SPMD per-core Bass program (identical on all 8 cores)."""
    assert sum(chunks) == s_total and all(c <= 256 for c in chunks)
    assert t_steps % DMA_BATCH == 0 and RING % DMA_BATCH == 0

    nc = bacc.Bacc("TRN2", target_bir_lowering=False, debug=False,
                   num_devices=N_CORES)

    # x duplicated on both partition halves:
    #   x_d[c, t*S + s] = x_d[64 + c, t*S + s] = x[s, t, c]
    x_d = nc.declare_dram_parameter("x", [128, t_steps * s_total], F16,
                                    isOutput=False)
    # wk duplicated over both partition halves: wk2[p,:] = Wk[p%64,:]
    wk_d = nc.declare_dram_parameter("wk", [128, 256], F16, isOutput=False)
    wr_d = nc.declare_dram_parameter("wr", [256, 256], F16, isOutput=False)
    # per-chain outputs: y{ch}[p, t, j, s_local] = w_t[u = j*128 + p, s]
    y_aps = [nc.declare_dram_parameter(f"y{ch}", [128, t_steps, 2, ncs],
                                       F16, isOutput=True).ap()
             for ch, ncs in enumerate(chunks)]
    x_ap, wk_ap, wr_ap = x_d.ap(), wk_d.ap(), wr_d.ap()

    nch = len(chunks)
    offs = [sum(chunks[:i]) for i in range(nch)]
    Tanh = mybir.ActivationFunctionType.Tanh
    MUL, ADD = mybir.AluOpType.mult, mybir.AluOpType.add

    with ExitStack() as ctx:
        tc = ctx.enter_context(tile.TileContext(nc))
        const = ctx.enter_context(tc.tile_pool(name="const", bufs=1))
        x_sb = const.tile([128, t_steps * s_total], F16)
        wk_sb = const.tile([128, 256], F16)
        wr0_sb = const.tile([128, 256], F16)
        wr1_sb = const.tile([128, 256], F16)
        junk = const.tile([128, 512], F16)
        # per-chain state rings: slot k at cols [k*2*ncs, (k+1)*2*ncs)
        rings = [const.tile([128, RING * 2 * chunks[ch]], F16,
                            name=f"wring{ch}") for ch in range(nch)]

        # wr0 first (heater dep), then wk + first x chunk (step-0 deps)
        nc.sync.dma_start(wr0_sb[:], wr_ap[0:128, :])
        nc.sync.dma_start(wk_sb[:], wk_ap[:])
        nc.sync.dma_start(wr1_sb[:], wr_ap[128:256, :])
        nc.vector.memset(junk[:], 0.0)
        for ch in range(nch):
            # init state = ring slot RING-1 (step 0 reads (0-1) % RING)
            ncs = chunks[ch]
            nc.vector.memset(rings[ch][:, (RING - 1) * 2 * ncs:], 0.0)
        def push_xchunk(t0, tlen):
            lo, hi = t0 * s_total, (t0 + tlen) * s_total
            nc.sync.dma_start(x_sb[:, lo:hi], x_ap[:, lo:hi])

        for t0, tlen, push_at in XCHUNKS:
            if push_at < 0:
                push_xchunk(t0, tlen)

        g_pool = ctx.enter_context(tc.tile_pool(name="g", bufs=4))
        ps_pool = ctx.enter_context(tc.tile_pool(name="ps", bufs=2,
                                                 space="PSUM"))
        heat_ps = ps_pool.tile([128, 512], F32, tag="heat", name="heat_ps")

        def heat(n):
            for _ in range(n):
                nc.tensor.matmul(heat_ps[:], wr0_sb[:, 0:128], junk[:],
                                 start=True, stop=True)

        def rslot(ch, t):
            ncs = chunks[ch]
            k = t % RING
            return rings[ch][:, k * 2 * ncs:(k + 1) * 2 * ncs]

        # initial heater burst: warms HAM while x streams in
        heat(heat_burst)

        def rec_group(ch, t, wp):
            """One chain-step accumulation group, emitted in iteration t
            (NO lookahead).  The two K=64 x matmuls open the group: with
            same-iteration emission their psum-free dependency is the tanh
            of step t-2 -- ~2 periods stale -- so they never block the PE
            FIFO (1-step-lookahead x-opens waited on tanh(t-1) and cost
            ~300ns/step of PE dead time).  They execute early, during this
            chain's blend wait, and the group STOP lands right after the
            4 rec matmuls, which is the true serial path."""
            ncs, off = chunks[ch], offs[ch]
            ps = ps_pool.tile([128, 2 * ncs], F32, tag=f"ps{ch}",
                              name=f"ps{ch}_{t}")
            xcol = t * s_total
            sl = slice(xcol + off, xcol + off + ncs)
            half = 64 * (ch % 2)
            nc.tensor.matmul(ps[:, 0:ncs],
                             wk_sb[half:half + 64, 0:128],
                             x_sb[half:half + 64, sl],
                             start=True, stop=False)
            nc.tensor.matmul(ps[:, ncs:2 * ncs],
                             wk_sb[half:half + 64, 128:256],
                             x_sb[half:half + 64, sl],
                             start=False, stop=False)
            nc.tensor.matmul(ps[:, 0:ncs], wr0_sb[:, 0:128],
                             wp[:, 0:ncs], start=False, stop=False)
            nc.tensor.matmul(ps[:, 0:ncs], wr1_sb[:, 0:128],
                             wp[:, ncs:2 * ncs], start=False, stop=False)
            nc.tensor.matmul(ps[:, ncs:2 * ncs], wr0_sb[:, 128:256],
                             wp[:, 0:ncs], start=False, stop=False)
            nc.tensor.matmul(ps[:, ncs:2 * ncs], wr1_sb[:, 128:256],
                             wp[:, ncs:2 * ncs], start=False, stop=True)
            return ps

        for t in range(t_steps):
            heat(heat_per_step)
            for t0, tlen, push_at in XCHUNKS:
                if push_at == t:
                    push_xchunk(t0, tlen)
            # output flush: batches of 4, except the last 4 steps go in
            # pairs so the final DMA has less left to drain at kernel end
            nb = DMA_BATCH if t < t_steps - 4 else 2
            flush = (t % nb == nb - 1) if t < t_steps - 4 else (t % 2 == 1)
            for ch in range(nch):
                ncs = chunks[ch]
                wp = rslot(ch, t - 1)
                ps = rec_group(ch, t, wp)

                g = g_pool.tile([128, 2 * ncs], F16, tag=f"g{ch}",
                                name=f"g{ch}_{t}")
                nc.scalar.activation(g[:], ps[:], Tanh)
                # fused blend: w_new = 0.1*w_old + g in ONE DVE op (the
                # split tensor_scalar+tensor_add pair cost ~580ns/chain and
                # delayed the blend behind unrelated DVE queue traffic).
                nc.vector.scalar_tensor_tensor(rslot(ch, t)[:], wp[:],
                                               1.0 - LEAKY, g[:], MUL, ADD)

                if flush:
                    k0 = (t - (nb - 1)) % RING
                    src = rings[ch][:, k0 * 2 * ncs:(k0 + nb) * 2 * ncs]
                    dst = y_aps[ch][:, t - (nb - 1):t + 1, :, :]
                    nc.sync.dma_start(dst, src)
                if t == 0:
                    # startup stagger: dep-free fillers delay the next
                    # chain's first group on the PE FIFO so the 3 chains
                    # phase-shift by ~ACT duration instead of running the
                    # whole scan in costly lockstep.
                    heat(2)

    nc.compile()
    return nc


_PROGRAM = None

# test-harness knob: when trace=True, the BassKernelResults (with
# exec_time_ns from neuron-profile) is stashed in PROFILE["last"].
PROFILE = {"trace": False, "last": None}


def _get_program():
    global _PROGRAM
    if _PROGRAM is None:
        _PROGRAM = build_program()
    return _PROGRAM


def _pack_x(xs, t_steps, s_total):
    """(S, T, C) fp32 -> packed (128, T*S) fp16, duplicated on both halves."""
    xt = np.ascontiguousarray(xs.transpose(2, 1, 0))      # (C, T, S)
    packed = np.empty((128, t_steps * s_total), np.float16)
    pv = packed.reshape(2, 64, t_steps * s_total)
    pv[0] = xt.reshape(64, -1)
    pv[1] = pv[0]
    return packed


def kernel(**inputs):
    x = np.asarray(inputs["inputs"], np.float32)          # (8,128,128,64)
    wsets = [
        (np.asarray(inputs["h_fwd_k"]), np.asarray(inputs["h_fwd_r"])),
        (np.asarray(inputs["h_bwd_k"]), np.asarray(inputs["h_bwd_r"])),
        (np.asarray(inputs["v_fwd_k"]), np.asarray(inputs["v_fwd_r"])),
        (np.asarray(inputs["v_bwd_k"]), np.asarray(inputs["v_bwd_r"])),
    ]
    nc = _get_program()

    in_maps = []
    for core in range(N_CORES):
        scan, bhalf = core // 2, core % 2
        xb = x[bhalf * 4:(bhalf + 1) * 4]                 # (4, NH, NW, C)
        if scan >= 2:                                     # vertical: cols as seqs
            xb = xb.transpose(0, 2, 1, 3)                 # (4, NW, NH, C)
        xs = xb.reshape(S, T, C)
        if scan % 2 == 1:                                 # bwd: reverse time
            xs = np.ascontiguousarray(xs[:, ::-1])
        wk, wr = wsets[scan]
        wk2 = np.concatenate([wk, wk], axis=0).astype(np.float16)   # (128,256)
        wr16 = (LEAKY * wr).astype(np.float16)                      # (256,256)
        in_maps.append({"x": _pack_x(xs, T, S), "wk": wk2, "wr": wr16})

    res = run_bass_kernel_spmd(nc, in_maps, list(range(N_CORES)),
                               trace=PROFILE["trace"])
    PROFILE["last"] = res
    results = res.results

    out = np.empty((B, NH, NW, 4 * U), np.float32)
    for core in range(N_CORES):
        scan, bhalf = core // 2, core % 2
        # concat per-chain outputs (128, T, 2, ncs) back to (128, T, 2, S)
        y = np.concatenate([results[core][f"y{ch}"]
                            for ch in range(len(CHUNKS))], axis=3)
        h = LEAKY * y.astype(np.float32)
        hs = h.transpose(3, 1, 2, 0).reshape(S, T, U)     # (s, t, u=(j,p))
        if scan % 2 == 1:
            hs = hs[:, ::-1]
        dst = out[bhalf * 4:(bhalf + 1) * 4, :, :, scan * U:(scan + 1) * U]
        if scan < 2:
            dst[:] = hs.reshape(4, NH, NW, U)
        else:
            dst[:] = hs.reshape(4, NW, NH, U).transpose(0, 2, 1, 3)
    return out

